# revision 37
# baseline (speedup 1.0000x reference)
"""Trainium2 Bass kernel for nn_L1RegressionActionHead.

Data-parallel over batch: 16 batch items -> 8 cores x 2 items.
All activations are dim-major on chip: (dims on partitions, tokens on the
free axis), so every matmul streams with the contraction dim on partitions.

RoPE: q/k projection weights are column-permuted on the host so each head's
128 dims are de-interleaved (even dims on partitions 0:64, odd on 64:128).
rotate_half is then a swap of the two 64-partition halves (2 SBUF->SBUF DMAs
issued from the idle gpsimd queue) and cos/sin become plain elementwise
multiplies.  1/sqrt(HD) is folded into the q tables, sigmoid(g) into the
k_task tables, the rotate sign into sin.

The q/o projections run as fp8(e4m3) DoubleRow matmuls (2 k-tiles per pass):
weights and x are quantized host-side (x*32, w*2048), the scale is folded
into the psum-consuming activation, and the o_proj output is carried as
64*y end-to-end (layernorm is scale-invariant; eps is scaled to match).

Softmax: |scores| < ~4 so exp needs no max subtraction.  The denominator is
summed+broadcast with a ones-matrix matmul and inverted with DVE
reciprocal_approx_fast (~51 ULP) - no Ln/Exp round trip, so the scalar
engine only ever uses the exp_and_others + sqrt_and_others activation tables
(2 table loads total instead of 73).

LayerNorm gamma/beta are folded into w_ffn/b_ffn on the host, so the kernel
only computes (y - mu) * rstd.  o_proj bias+residual is one fused DVE
scalar_tensor_tensor.  Stages are emitted so attention (ScalarE/DVE heavy)
overlaps the next projection's matmuls and the PE never idles long enough
to re-throttle (HAM).
"""

import math
import sys

import numpy as np

sys.path.insert(0, "/opt/trn_rl_repo")

import ml_dtypes  # noqa: E402

import concourse.bass as bass  # noqa: E402
import concourse.tile as tile  # noqa: E402
from concourse import bacc, mybir  # noqa: E402
from concourse.bass_utils import run_bass_kernel_spmd  # noqa: E402

BF16 = ml_dtypes.bfloat16
FP8 = ml_dtypes.float8_e4m3fn  # matches TRN float8e4 bit layout for |v|<=240
F32 = mybir.dt.float32
BF = mybir.dt.bfloat16
AF = mybir.ActivationFunctionType
F8D = mybir.dt.float8e4
OP = mybir.AluOpType

DIM = 1024
HEADS = 8
HD = 128
B = 16
T = 1024
KT = 64
KA = 2
KV = KT + KA  # 66
LN_EPS = 1e-5
NCORES = 8
BPC = B // NCORES  # 2 batch items per core
P = 128
TK = DIM // P  # 8 k/d tiles
NCH = T // 512  # 2 free-dim chunks of 512 tokens
F8 = None  # set below (mybir.dt.float8e4)
XS8 = 32.0     # fp8 scale for x
WS8 = 2048.0   # fp8 scale for q weights
QSCALE = 1.0 / (XS8 * WS8)  # folded into the q identity activation
OSC = 64.0     # attention-output fp8 scale; y is carried as 64*y (LN-invariant)

# de-interleave: even dims on partitions 0:64, odd dims on 64:128, so
# rotate_half is a swap of the two 64-partition halves (2 SBUF->SBUF DMAs)
# with the sign folded into the sin table.
_PERM_HEAD = np.concatenate([np.arange(0, HD, 2), np.arange(1, HD, 2)])
_SIGN_HEAD = np.concatenate([-np.ones(64, np.float32), np.ones(64, np.float32)])
_PERM_FULL = np.concatenate([h * HD + _PERM_HEAD for h in range(HEADS)])

# weight order inside the "wcat" input tensor
_WIDX = {"w_qa": 0, "w_qt": 1, "w_ka": 2, "w_kt": 3, "w_va": 4, "w_vt": 5,
         "w_o": 6, "w_ffn": 7}
# bias slots inside "bias_cat": per-partition [128, slot, ko]
_BIDX = {"b_qa": 0, "b_qt": 1, "b_ka": 2, "b_kt": 3, "b_o": 4, "b_ffn": 5}

_CACHED = None  # compiled Bass program, built once per process
LAST_RESULTS = None  # BassKernelResults of the most recent run


def _build_program():
    nc = bacc.Bacc("TRN2", target_bir_lowering=False, debug=False,
                   enable_asserts=False)

    xt_d = nc.dram_tensor("xt", (P, BPC, TK, T), BF, kind="ExternalInput").ap()
    xt8_d = nc.dram_tensor("xt8", (P, BPC, TK, T), F8D, kind="ExternalInput").ap()
    wq8_d = nc.dram_tensor("wq8", (3, P, TK, DIM), F8D, kind="ExternalInput").ap()
    hcat_d = nc.dram_tensor("hcat", (P, TK, 2 * KV), BF, kind="ExternalInput").ap()
    wcat_d = nc.dram_tensor("wcat", (8, P, TK, DIM), BF, kind="ExternalInput").ap()
    bias_d = nc.dram_tensor("bias_cat", (P, 6, TK), F32, kind="ExternalInput").ap()
    bv_d = nc.dram_tensor("bv_comb", (P, DIM), BF, kind="ExternalInput").ap()
    vsel_d = nc.dram_tensor("vsel", (P, KV), BF, kind="ExternalInput").ap()
    cosq_d = nc.dram_tensor("cosq", (P, T), BF, kind="ExternalInput").ap()
    sinq_d = nc.dram_tensor("sinq", (P, T), BF, kind="ExternalInput").ap()
    cosk_d = nc.dram_tensor("cosk", (P, 2 * KV), BF, kind="ExternalInput").ap()
    sink_d = nc.dram_tensor("sink", (P, 2 * KV), BF, kind="ExternalInput").ap()
    out_d = nc.dram_tensor("outt", (P, BPC, TK, T), BF, kind="ExternalOutput").ap()

    with tile.TileContext(nc) as tc:
        _trace(nc, tc, xt_d, xt8_d, wq8_d, hcat_d, wcat_d, bias_d, bv_d,
               vsel_d, cosq_d, sinq_d, cosk_d, sink_d, out_d)
    nc.compile()
    return nc


def _trace(nc, tc, xt_d, xt8_d, wq8_d, hcat_d, wcat_d, bias_d, bv_d,
           vsel_d, cosq_d, sinq_d, cosk_d, sink_d, out_d):
    import contextlib
    ctx = contextlib.ExitStack()
    with ctx:
        consts = ctx.enter_context(tc.tile_pool(name="consts", bufs=1))
        acts = ctx.enter_context(tc.tile_pool(name="acts", bufs=1))
        qpool = ctx.enter_context(tc.tile_pool(name="qpool", bufs=4))
        wpool = ctx.enter_context(tc.tile_pool(name="wpool", bufs=2))
        swp = ctx.enter_context(tc.tile_pool(name="swp", bufs=4))
        q8p = ctx.enter_context(tc.tile_pool(name="q8p", bufs=2))
        sb512 = ctx.enter_context(tc.tile_pool(name="sb512", bufs=3))
        rcp_p = ctx.enter_context(tc.tile_pool(name="rcpp", bufs=1))
        f32p = ctx.enter_context(tc.tile_pool(name="f32p", bufs=2))
        psum = ctx.enter_context(tc.tile_pool(name="psum", bufs=4, space="PSUM"))
        pacc = ctx.enter_context(tc.tile_pool(name="pacc", bufs=2, space="PSUM"))

        def load_w(wname):
            wt = wpool.tile([P, TK, DIM], BF, tag="w", name=wname)
            nc.sync.dma_start(wt[:, :, :], wcat_d[_WIDX[wname], :, :, :])
            return wt

        # ---- DMAs in need order (first loads chunked per k-tile so the
        #      k-task matmuls can start as soon as slice 0 lands) ---------
        hcat_sb = consts.tile([P, TK, 2 * KV], BF, tag="hcat")
        nc.sync.dma_start(hcat_sb[:], hcat_d[:])
        bias_sb = consts.tile([P, 6, TK], F32, tag="bias")
        nc.sync.dma_start(bias_sb[:], bias_d[:])
        wkt = wpool.tile([P, TK, DIM], BF, tag="w", name="w_kt")
        for k in range(TK):
            nc.sync.dma_start(wkt[:, k, :], wcat_d[_WIDX["w_kt"], :, k, :])
        wq8a = q8p.tile([P, TK, DIM], F8D, tag="q8", name="wq8a")
        xt8_sb = acts.tile([P, BPC, TK, T], F8D, tag="xt8")
        for k in range(TK):
            nc.sync.dma_start(wq8a[:, k, :], wq8_d[0, :, k, :])
            nc.sync.dma_start(xt8_sb[:, 0, k, :], xt8_d[:, 0, k, :])
        cosq_sb = consts.tile([P, T], BF, tag="cosq")
        nc.sync.dma_start(cosq_sb[:], cosq_d[:])
        sinq_sb = consts.tile([P, T], BF, tag="sinq")
        nc.sync.dma_start(sinq_sb[:], sinq_d[:])
        wka = load_w("w_ka")
        nc.sync.dma_start(xt8_sb[:, 1], xt8_d[:, 1])
        wq8t = q8p.tile([P, TK, DIM], F8D, tag="q8", name="wq8t")
        nc.sync.dma_start(wq8t[:], wq8_d[1])
        cosk_sb = consts.tile([P, 2 * KV], BF, tag="cosk")
        nc.sync.dma_start(cosk_sb[:], cosk_d[:])
        sink_sb = consts.tile([P, 2 * KV], BF, tag="sink")
        nc.sync.dma_start(sink_sb[:], sink_d[:])
        bv_sb = consts.tile([P, DIM], BF, tag="bv")
        nc.sync.dma_start(bv_sb[:], bv_d[:])
        vsel = consts.tile([P, KV], BF, tag="vsel")
        nc.sync.dma_start(vsel[:], vsel_d[:])
        xt_sb = acts.tile([P, BPC, TK, T], BF, tag="xt")
        nc.sync.dma_start(xt_sb[:, 0], xt_d[:, 0])
        nc.sync.dma_start(xt_sb[:, 1], xt_d[:, 1])
        ones_mat = consts.tile([P, P], BF, tag="onesm")
        nc.vector.memset(ones_mat[:], 1.0)
        # scaled ones: softmax denominator picks up 1/OSC so the normalized
        # attention output comes out pre-scaled by OSC for the fp8 store
        sc_ones = consts.tile([P, P], BF, tag="sconem")
        nc.vector.memset(sc_ones[:], 1.0 / OSC)
        eps_sb = consts.tile([P, 1], F32, tag="eps")
        nc.vector.memset(eps_sb[:], LN_EPS * OSC * OSC)

        def bias_ap(bname, n):
            return bias_sb[:, _BIDX[bname], n:n + 1]

        def swap_halves(dst, sw, width):
            # rotate_half: swap the two 64-partition blocks via 2 DMAs,
            # issued from two idle queues so the issues overlap
            nc.gpsimd.dma_start(sw[0:64, 0:width], dst[64:128, 0:width])
            nc.sync.dma_start(sw[64:128, 0:width], dst[0:64, 0:width])

        def rope_q(dst):
            # dst: (128, T) bf16, in-place
            sw = swp.tile([P, T], BF, tag="sw")
            swap_halves(dst, sw[:, :], T)
            nc.vector.tensor_mul(sw[:], sw[:], sinq_sb[:])
            nc.vector.tensor_mul(dst, dst, cosq_sb[:])
            nc.vector.tensor_add(dst, dst, sw[:])

        DR = mybir.MatmulPerfMode.DoubleRow

        def q_mm(qt_t, w8, bname, b, n):
            # fp8 DoubleRow: contract 2 k-tiles per pass (K=256 virtual)
            for c in range(NCH):
                cs = slice(c * 512, (c + 1) * 512)
                ps = psum.tile([P, 512], F32, tag="ps")
                for kp in range(TK // 2):
                    nc.tensor.matmul(
                        ps[:], w8[:, 2 * kp:2 * kp + 2, n * P:(n + 1) * P],
                        xt8_sb[:, b, 2 * kp:2 * kp + 2, cs],
                        start=(kp == 0), stop=(kp == TK // 2 - 1),
                        perf_mode=DR)
                nc.scalar.activation(
                    qt_t[:, n, cs], ps[:], AF.Identity,
                    bias=bias_ap(bname, n), scale=QSCALE)
            rope_q(qt_t[:, n, :])

        # ================= k_task projection ===========================
        # krot columns: [0:64]=task b0, [64:128]=task b1, [128:130]=ad b0,
        # [130:132]=ad b1
        krot = acts.tile([P, TK, 2 * KV], BF, tag="krot")
        for n in range(TK):
            ps = psum.tile([P, 512], F32, tag="ps")
            for k in range(TK):
                nc.tensor.matmul(ps[:, 0:128], wkt[:, k, n * P:(n + 1) * P],
                                 hcat_sb[:, k, 0:128],
                                 start=(k == 0), stop=(k == TK - 1))
            nc.scalar.activation(krot[:, n, 0:128], ps[:, 0:128],
                                 AF.Identity, bias=bias_ap("b_kt", n), scale=1.0)

        # ================= q_adapter b0 ================================
        q_rot = {}  # (qi, b) -> (128, TK, T) bf16, qi: 0=adapter 1=task
        qa0 = qpool.tile([P, TK, T], BF, tag="qbuf", name="qa0")
        q_rot[(0, 0)] = qa0
        for n in range(TK):
            q_mm(qa0, wq8a, "b_qa", 0, n)

        # ================= k_adapter + k rope ==========================
        for n in range(TK):
            ps = psum.tile([P, 512], F32, tag="ps")
            for k in range(TK):
                nc.tensor.matmul(ps[:, 128:132], wka[:, k, n * P:(n + 1) * P],
                                 hcat_sb[:, k, 128:132],
                                 start=(k == 0), stop=(k == TK - 1))
            nc.scalar.activation(krot[:, n, 128:132], ps[:, 128:132],
                                 AF.Identity, bias=bias_ap("b_ka", n), scale=1.0)

        wvt = load_w("w_vt")  # into slot freed by wkt
        wva = load_w("w_va")  # into slot freed by wka

        # ================= v projections (token-major) =================
        # vcomb rows: [0:64]=task tokens, [64:66]=adapter tokens; the bias
        # lands via rank-1 matmuls (vsel x bv) so the v pipeline only
        # depends on the PE + ScalarE.  Task loop first: wva arrives a few
        # us after wvt (it waits on wka's pool slot).
        vcomb = acts.tile([P, BPC, DIM], BF, tag="vcomb")
        for b in range(BPC):
            for c in range(NCH):
                cs = slice(c * 512, (c + 1) * 512)
                ps = psum.tile([P, 512], F32, tag="ps")
                for k in range(TK):
                    nc.tensor.matmul(ps[0:64, :],
                                     hcat_sb[:, k, b * 64:(b + 1) * 64],
                                     wvt[:, k, cs],
                                     start=(k == 0), stop=(k == TK - 1))
                nc.tensor.matmul(ps[0:64, :], vsel[0:1, 0:KT], bv_sb[0:1, cs],
                                 start=False, stop=True, skip_group_check=True)
                nc.scalar.activation(vcomb[0:64, b, cs], ps[0:64, :],
                                     AF.Identity, scale=1.0)
        for b in range(BPC):
            for c in range(NCH):
                cs = slice(c * 512, (c + 1) * 512)
                ps = psum.tile([P, 512], F32, tag="ps")
                for k in range(TK):
                    nc.tensor.matmul(ps[64:66, :],
                                     hcat_sb[:, k, 128 + 2 * b:130 + 2 * b],
                                     wva[:, k, cs],
                                     start=(k == 0), stop=(k == TK - 1))
                nc.tensor.matmul(ps[64:66, :], vsel[32:33, KT:KV],
                                 bv_sb[32:33, cs],
                                 start=False, stop=True, skip_group_check=True)
                nc.scalar.activation(vcomb[64:66, b, cs], ps[64:66, :],
                                     AF.Identity, scale=1.0)

        # ================= q_task b0 ===================================
        qt0 = qpool.tile([P, TK, T], BF, tag="qbuf", name="qt0")
        q_rot[(1, 0)] = qt0
        for n in range(TK):
            q_mm(qt0, wq8t, "b_qt", 0, n)

        # k rope (deferred: qt0's ropes gate T5, so they go first on DVE)
        for n in range(TK):
            sw = rcp_p.tile([P, 2 * KV], BF, tag="ksw")
            swap_halves(krot[:, n, :], sw[:, :], 2 * KV)
            nc.vector.tensor_mul(sw[:], sw[:], sink_sb[:])
            nc.vector.tensor_mul(krot[:, n, :], krot[:, n, :], cosk_sb[:])
            nc.vector.tensor_add(krot[:, n, :], krot[:, n, :], sw[:])

        wo8 = q8p.tile([P, TK, DIM], F8D, tag="q8", name="wo8")
        nc.sync.dma_start(wo8[:], wq8_d[2])

        # ---- attention helpers ----------------------------------------
        attn = {}

        def attn_scores(b, h):
            out = []
            for c in range(NCH):
                cs = slice(c * 512, (c + 1) * 512)
                scps = psum.tile([P, 512], F32, tag="ps")
                nc.tensor.matmul(scps[0:64, :], krot[:, h, b * 64:(b + 1) * 64],
                                 q_rot[(1, b)][:, h, cs], start=True, stop=True)
                nc.tensor.matmul(scps[64:66, :],
                                 krot[:, h, 128 + 2 * b:130 + 2 * b],
                                 q_rot[(0, b)][:, h, cs], start=True, stop=True)
                ex = sb512.tile([P, 512], BF, tag="s", name="ex")
                nc.scalar.activation(ex[0:KV, :], scps[0:KV, :], AF.Exp)
                out.append(ex)
            return out

        def attn_finish(b, h, exs):
            at = attn[b]
            for c in range(NCH):
                cs = slice(c * 512, (c + 1) * 512)
                ex = exs[c]
                dnps = psum.tile([P, 512], F32, tag="ps")
                nc.tensor.matmul(dnps[:], sc_ones[0:KV, :],
                                 ex[0:KV, :], start=True, stop=True)
                ovps = psum.tile([P, 512], F32, tag="ps")
                nc.tensor.matmul(ovps[:], vcomb[0:KV, b, h * P:(h + 1) * P],
                                 ex[0:KV, :], start=True, stop=True)
                rcb = f32p.tile([P, 512], F32, tag="f32", name="rcb")
                nc.vector.reciprocal_approx_fast(rcb[:], dnps[:])
                nc.vector.tensor_mul(at[:, h, cs], ovps[:], rcb[:])

        # ================= q_adapter b1 ================================
        # (PE cover for the qt0 rope tail; its own ropes drain under T5)
        qa1 = qpool.tile([P, TK, T], BF, tag="qbuf", name="qa1")
        q_rot[(0, 1)] = qa1
        for n in range(TK):
            q_mm(qa1, wq8a, "b_qa", 1, n)

        # ============ T5: attention b0 (x) q_task b1 ===================
        attn[0] = acts.tile([P, TK, T], F8D, tag="attn0", name="attn0")
        qt1 = qpool.tile([P, TK, T], BF, tag="qbuf", name="qt1")
        q_rot[(1, 1)] = qt1
        prev = None
        for h in range(HEADS):
            exs = attn_scores(0, h)
            if prev is not None:
                attn_finish(0, h - 1, prev)
            prev = exs
            q_mm(qt1, wq8t, "b_qt", 1, h)
        attn_finish(0, HEADS - 1, prev)

        wffn = load_w("w_ffn")  # into slot freed by wva

        # ============ T6: o_proj b0 (x) attention b1 (lag-2) ===========
        def oproj_tile(b, n, y):
            # fp8 DoubleRow o_proj; psum = OSC*WS8*(attn@wo), so scale by
            # 1/WS8 and add OSC*(x + b_o) (b_o folded into x on the host)
            for c in range(NCH):
                cs = slice(c * 512, (c + 1) * 512)
                ps = psum.tile([P, 512], F32, tag="ps")
                for kp in range(TK // 2):
                    nc.tensor.matmul(
                        ps[:], wo8[:, 2 * kp:2 * kp + 2, n * P:(n + 1) * P],
                        attn[b][:, 2 * kp:2 * kp + 2, cs],
                        start=(kp == 0), stop=(kp == TK // 2 - 1),
                        perf_mode=DR)
                nc.vector.scalar_tensor_tensor(
                    y[:, n, cs], ps[:], 1.0 / WS8,
                    xt_sb[:, b, n, cs], OP.mult, OP.add)

        attn[1] = acts.tile([P, TK, T], F8D, tag="attn1", name="attn1")
        y0 = qpool.tile([P, TK, T], BF, tag="qbuf", name="y0")
        pend = {}
        for h in range(HEADS):
            oproj_tile(0, h, y0)
            if h >= 2:  # lag so the b1 q-rope tail drains under o_proj
                pend[h - 2] = attn_scores(1, h - 2)
            if h >= 3:
                attn_finish(1, h - 3, pend.pop(h - 3))
        for h in range(HEADS - 2, HEADS):
            pend[h] = attn_scores(1, h)
        for h in range(HEADS - 3, HEADS):
            attn_finish(1, h, pend.pop(h))

        # ---- layernorm (folded: yn = (y - mu) * rstd) -----------------
        def ln_stats_tile(sps, qps, y, n):
            for c in range(NCH):
                cs = slice(c * 512, (c + 1) * 512)
                ysq = sb512.tile([P, 512], BF, tag="s", name=f"ysq{n}{c}")
                nc.scalar.activation(ysq[:], y[:, n, cs], AF.Square)
                nc.tensor.matmul(sps[:, cs], ones_mat[:], y[:, n, cs],
                                 start=(n == 0), stop=(n == TK - 1),
                                 skip_group_check=True)
                nc.tensor.matmul(qps[:, cs], ones_mat[:], ysq[:],
                                 start=(n == 0), stop=(n == TK - 1),
                                 skip_group_check=True)

        def ln_apply(b, y, yn, sps, qps):
            # rstd_bc/mrs_bc: (128, T) bf16, broadcast over partitions
            rstd_bc = swp.tile([P, T], BF, tag="sw", name=f"rstd{b}")
            mrs_bc = swp.tile([P, T], BF, tag="sw", name=f"mrs{b}")
            for c in range(NCH):
                cs = slice(c * 512, (c + 1) * 512)
                mu = sb512.tile([P, 512], BF, tag="s", name=f"mu{b}{c}")
                nc.vector.tensor_scalar_mul(mu[:], sps[:, cs], 1.0 / DIM)
                m2 = sb512.tile([P, 512], BF, tag="s", name=f"m2{b}{c}")
                nc.vector.tensor_mul(m2[:], mu[:], mu[:])
                vq = f32p.tile([P, 512], F32, tag="f32")
                nc.vector.scalar_tensor_tensor(
                    vq[:], qps[:, cs], 1.0 / DIM, m2[:], OP.mult, OP.subtract)
                sdev = f32p.tile([P, 512], F32, tag="f32", name=f"sd{b}{c}")
                nc.scalar.activation(sdev[:], vq[:], AF.Sqrt, bias=eps_sb[:],
                                     scale=1.0)
                rstd_f = f32p.tile([P, 512], F32, tag="f32", name=f"rf{b}{c}")
                nc.vector.reciprocal_approx_fast(rstd_f[:], sdev[:])
                nc.vector.tensor_scalar_mul(rstd_bc[:, cs], rstd_f[:], 1.0)
                nc.vector.tensor_mul(mrs_bc[:, cs], mu[:], rstd_bc[:, cs])
            for n in range(TK):
                nc.vector.tensor_mul(yn[:, n, :], y[:, n, :], rstd_bc[:])
                nc.vector.tensor_tensor(yn[:, n, :], yn[:, n, :], mrs_bc[:],
                                        OP.subtract)

        def ffn_tile(b, n, yn):
            for c in range(NCH):
                cs = slice(c * 512, (c + 1) * 512)
                ps = psum.tile([P, 512], F32, tag="ps")
                for k in range(TK):
                    nc.tensor.matmul(ps[:], wffn[:, k, n * P:(n + 1) * P],
                                     yn[:, k, cs],
                                     start=(k == 0), stop=(k == TK - 1))
                ob = sb512.tile([P, 512], BF, tag="s", name=f"ob{b}{n}{c}")
                nc.scalar.activation(ob[:], ps[:], AF.Relu,
                                     bias=bias_ap("b_ffn", n), scale=1.0)
                nc.sync.dma_start(out_d[:, b, n, cs], ob[:])

        # ============ T7: LN b0 stats, o_proj b1, ln_apply(0) ==========
        y1 = qpool.tile([P, TK, T], BF, tag="qbuf", name="y1")
        sps0 = pacc.tile([P, T], F32, tag="acc", name="sps0")
        qps0 = pacc.tile([P, T], F32, tag="acc", name="qps0")
        for n in range(TK):
            ln_stats_tile(sps0, qps0, y0, n)
        for n in range(TK):
            oproj_tile(1, n, y1)
        yn0 = qpool.tile([P, TK, T], BF, tag="qbuf", name="yn0")
        ln_apply(0, y0, yn0, sps0, qps0)

        # ============ T8: LN b1 stats, ln_apply(1), ffn b0 =============
        sps1 = pacc.tile([P, T], F32, tag="acc", name="sps1")
        qps1 = pacc.tile([P, T], F32, tag="acc", name="qps1")
        for n in range(TK):
            ln_stats_tile(sps1, qps1, y1, n)
        yn1 = qpool.tile([P, TK, T], BF, tag="qbuf", name="yn1")
        ln_apply(1, y1, yn1, sps1, qps1)
        for n in range(TK):
            ffn_tile(0, n, yn0)

        # ============ T9: ffn b1 =======================================
        for n in range(TK):
            ffn_tile(1, n, yn1)


# =====================  host-side preparation  =========================

def _rope_tables(L):
    inv = 1.0 / (10000.0 ** (np.arange(0, HD, 2, dtype=np.float32) / HD))
    freqs = np.arange(L, dtype=np.float32)[:, None] * inv[None, :]
    emb = np.concatenate([freqs, freqs], axis=-1)  # (L, 128)
    return np.cos(emb), np.sin(emb)


def _perm_tables(L, scale):
    cos, sin = _rope_tables(L)  # (L, 128)
    cosP = (cos[:, _PERM_HEAD].T * scale).astype(np.float32)      # (128, L)
    sinN = (sin[:, _PERM_HEAD].T * _SIGN_HEAD[:, None] * scale).astype(np.float32)
    return cosP, sinN


def _w_sb(w, permute):
    # (1024 k, 1024 n) -> (128 p, 8 ko, 1024 n) bf16, optional column perm
    if permute:
        w = w[:, _PERM_FULL]
    return np.ascontiguousarray(
        w.reshape(TK, P, DIM).transpose(1, 0, 2)).astype(BF16)


def _b_slot(bvec, permute):
    if permute:
        bvec = bvec[_PERM_FULL]
    return bvec.reshape(TK, P).T  # (128, 8)


def kernel(**inputs):
    global _CACHED
    if _CACHED is None:
        _CACHED = _build_program()
    nc = _CACHED

    inp = {k: np.asarray(v) for k, v in inputs.items()}
    x = inp["x"].astype(np.float32)
    h_a = inp["h_a"].astype(np.float32)
    h_t = inp["h_t"].astype(np.float32)
    p_in = inp["p"].astype(np.float32)
    ratio = 1.0 / (1.0 + np.exp(-np.float32(inp["g"][0])))  # sigmoid

    # fold layernorm gamma/beta into the ffn weights
    w_ffn = inp["ln_g"].astype(np.float32)[:, None] * inp["w_ffn"].astype(np.float32)
    b_ffn = inp["b_ffn"].astype(np.float32) + (
        inp["ln_b"].astype(np.float32) @ inp["w_ffn"].astype(np.float32))

    # weights (shared across cores)
    wcat = np.stack([
        _w_sb(inp["w_qa"], True), _w_sb(inp["w_qt"], True),
        _w_sb(inp["w_ka"], True), _w_sb(inp["w_kt"], True),
        _w_sb(inp["w_va"], False), _w_sb(inp["w_vt"], False),
        _w_sb(inp["w_o"], False), _w_sb(w_ffn, False)])

    def _w8(w, permute):
        wp = w.astype(np.float32) * WS8
        if permute:
            wp = wp[:, _PERM_FULL]
        wp = np.clip(wp, -240.0, 240.0)
        return np.ascontiguousarray(
            wp.reshape(TK, P, DIM).transpose(1, 0, 2)).astype(FP8)

    wq8 = np.stack([_w8(inp["w_qa"], True), _w8(inp["w_qt"], True),
                    _w8(inp["w_o"], False)])
    bias_cat = np.stack([
        _b_slot(inp["b_qa"], True), _b_slot(inp["b_qt"], True),
        _b_slot(inp["b_ka"], True), _b_slot(inp["b_kt"], True),
        _b_slot(inp["b_o"], False) * 0.0, _b_slot(b_ffn, False)],
        axis=1).astype(np.float32)  # (128, 6slots, 8ko)
    bv_comb = np.zeros((P, DIM), np.float32)
    bv_comb[0, :] = inp["b_vt"]
    bv_comb[32, :] = inp["b_va"]  # partition 32: legal matmul base partition
    bv_comb = bv_comb.astype(BF16)
    vsel = np.zeros((P, KV), np.float32)
    vsel[0, 0:KT] = 1.0
    vsel[32, KT:KV] = 1.0
    vsel = vsel.astype(BF16)

    cosq, sinq = _perm_tables(T, np.float32(1.0 / math.sqrt(HD)))
    coskt, sinkt = _perm_tables(KT, ratio)
    coska, sinka = _perm_tables(KA, np.float32(1.0))
    cosk = np.concatenate([coskt, coskt, coska, coska], axis=1)  # (128, 132)
    sink = np.concatenate([sinkt, sinkt, sinka, sinka], axis=1)

    shared = {
        "wcat": wcat, "wq8": wq8, "bias_cat": bias_cat, "bv_comb": bv_comb,
        "vsel": vsel,
        "cosq": cosq.astype(BF16), "sinq": sinq.astype(BF16),
        "cosk": cosk.astype(BF16), "sink": sink.astype(BF16),
    }

    in_maps = []
    for core in range(NCORES):
        b0 = core * BPC
        xc = x[b0:b0 + BPC]  # (2, 1024, 1024)
        xtf = np.ascontiguousarray(
            xc.reshape(BPC, T, TK, P).transpose(3, 0, 2, 1))  # (128,2,8,1024)
        xtr = xtf + inp["b_o"].astype(np.float32).reshape(TK, P).T[:, None, :, None]
        xt = (xtr * OSC).astype(BF16)  # o_proj residual: OSC*(x + b_o)
        xt8 = np.clip(xtf * XS8, -240.0, 240.0).astype(FP8)
        hcat = np.zeros((P, TK, 2 * KV), np.float32)
        for b in range(BPC):
            htT = h_t[b0 + b].T.reshape(TK, P, KT).transpose(1, 0, 2)
            hcat[:, :, b * KT:(b + 1) * KT] = htT
            had = np.stack([h_a[b0 + b, 0], p_in[b0 + b, 0]], axis=1)  # (1024,2)
            hcat[:, :, 2 * KT + b * KA:2 * KT + (b + 1) * KA] = (
                had.reshape(TK, P, KA).transpose(1, 0, 2))
        in_maps.append({"xt": xt, "xt8": xt8, "hcat": hcat.astype(BF16),
                        **shared})

    res = run_bass_kernel_spmd(nc, in_maps, core_ids=list(range(NCORES)))
    global LAST_RESULTS
    LAST_RESULTS = res

    out = np.empty((B, T, DIM), np.float32)
    for core in range(NCORES):
        ot = res.results[core]["outt"]  # (128, 2, 8, 1024) bf16
        out[core * BPC:(core + 1) * BPC] = (
            ot.astype(np.float32).transpose(1, 3, 2, 0).reshape(BPC, T, DIM))
    return out


# revision 38
# speedup vs baseline: 1.1004x; 1.1004x over previous
"""Trainium2 Bass kernel for nn_L1RegressionActionHead.

Data-parallel over batch: 16 batch items -> 8 cores x 2 items.
All activations are dim-major on chip: (dims on partitions, tokens on the
free axis), so every matmul streams with the contraction dim on partitions.

RoPE: q/k projection weights are column-permuted on the host so each head's
128 dims are de-interleaved (even dims on partitions 0:64, odd on 64:128).
rotate_half is then a swap of the two 64-partition halves (2 SBUF->SBUF DMAs
issued from the idle gpsimd queue) and cos/sin become plain elementwise
multiplies.  1/sqrt(HD) is folded into the q tables, sigmoid(g) into the
k_task tables, the rotate sign into sin.

The q/o projections run as fp8(e4m3) DoubleRow matmuls (2 k-tiles per pass):
weights and x are quantized host-side (x*32, w*2048), the scale is folded
into the psum-consuming activation, and the o_proj output is carried as
64*y end-to-end (layernorm is scale-invariant; eps is scaled to match).

Softmax: |scores| < ~4 so exp needs no max subtraction.  The denominator is
summed+broadcast with a ones-matrix matmul and inverted with DVE
reciprocal_approx_fast (~51 ULP) - no Ln/Exp round trip, so the scalar
engine only ever uses the exp_and_others + sqrt_and_others activation tables
(2 table loads total instead of 73).

LayerNorm gamma/beta are folded into w_ffn/b_ffn on the host, so the kernel
only computes (y - mu) * rstd.  o_proj bias+residual is one fused DVE
scalar_tensor_tensor.  Stages are emitted so attention (ScalarE/DVE heavy)
overlaps the next projection's matmuls and the PE never idles long enough
to re-throttle (HAM).
"""

import math
import sys

import numpy as np

sys.path.insert(0, "/opt/trn_rl_repo")

import ml_dtypes  # noqa: E402

import concourse.bass as bass  # noqa: E402
import concourse.tile as tile  # noqa: E402
from concourse import bacc, mybir  # noqa: E402
from concourse.bass_utils import run_bass_kernel_spmd  # noqa: E402

BF16 = ml_dtypes.bfloat16
FP8 = ml_dtypes.float8_e4m3fn  # matches TRN float8e4 bit layout for |v|<=240
F32 = mybir.dt.float32
BF = mybir.dt.bfloat16
AF = mybir.ActivationFunctionType
F8D = mybir.dt.float8e4
OP = mybir.AluOpType

DIM = 1024
HEADS = 8
HD = 128
B = 16
T = 1024
KT = 64
KA = 2
KV = KT + KA  # 66
LN_EPS = 1e-5
NCORES = 8
BPC = B // NCORES  # 2 batch items per core
P = 128
TK = DIM // P  # 8 k/d tiles
NCH = T // 512  # 2 free-dim chunks of 512 tokens
F8 = None  # set below (mybir.dt.float8e4)
XS8 = 32.0     # fp8 scale for x
WS8 = 2048.0   # fp8 scale for q weights
QSCALE = 1.0 / (XS8 * WS8)  # folded into the q identity activation
OSC = 64.0     # attention-output fp8 scale; y is carried as 64*y (LN-invariant)

# de-interleave: even dims on partitions 0:64, odd dims on 64:128, so
# rotate_half is a swap of the two 64-partition halves (2 SBUF->SBUF DMAs)
# with the sign folded into the sin table.
_PERM_HEAD = np.concatenate([np.arange(0, HD, 2), np.arange(1, HD, 2)])
_SIGN_HEAD = np.concatenate([-np.ones(64, np.float32), np.ones(64, np.float32)])
_PERM_FULL = np.concatenate([h * HD + _PERM_HEAD for h in range(HEADS)])

# weight order inside the "wcat" input tensor
_WIDX = {"w_qa": 0, "w_qt": 1, "w_ka": 2, "w_kt": 3, "w_va": 4, "w_vt": 5,
         "w_o": 6, "w_ffn": 7}
# bias slots inside "bias_cat": per-partition [128, slot, ko]
_BIDX = {"b_qa": 0, "b_qt": 1, "b_ka": 2, "b_kt": 3, "b_o": 4, "b_ffn": 5}

_CACHED = None  # compiled Bass program, built once per process
LAST_RESULTS = None  # BassKernelResults of the most recent run


def _build_program():
    nc = bacc.Bacc("TRN2", target_bir_lowering=False, debug=False,
                   enable_asserts=False)

    xt_d = nc.dram_tensor("xt", (P, BPC, TK, T), BF, kind="ExternalInput").ap()
    xt8_d = nc.dram_tensor("xt8", (P, BPC, TK, T), F8D, kind="ExternalInput").ap()
    wq8_d = nc.dram_tensor("wq8", (3, P, TK, DIM), F8D, kind="ExternalInput").ap()
    hcat_d = nc.dram_tensor("hcat", (P, TK, 2 * KV), BF, kind="ExternalInput").ap()
    wcat_d = nc.dram_tensor("wcat", (8, P, TK, DIM), BF, kind="ExternalInput").ap()
    bias_d = nc.dram_tensor("bias_cat", (P, 6, TK), F32, kind="ExternalInput").ap()
    bv_d = nc.dram_tensor("bv_comb", (P, DIM), BF, kind="ExternalInput").ap()
    vsel_d = nc.dram_tensor("vsel", (P, KV), BF, kind="ExternalInput").ap()
    cosq_d = nc.dram_tensor("cosq", (P, T), BF, kind="ExternalInput").ap()
    sinq_d = nc.dram_tensor("sinq", (P, T), BF, kind="ExternalInput").ap()
    cosk_d = nc.dram_tensor("cosk", (P, 2 * KV), BF, kind="ExternalInput").ap()
    sink_d = nc.dram_tensor("sink", (P, 2 * KV), BF, kind="ExternalInput").ap()
    out_d = nc.dram_tensor("outt", (P, BPC, TK, T), BF, kind="ExternalOutput").ap()

    with tile.TileContext(nc) as tc:
        _trace(nc, tc, xt_d, xt8_d, wq8_d, hcat_d, wcat_d, bias_d, bv_d,
               vsel_d, cosq_d, sinq_d, cosk_d, sink_d, out_d)
    nc.compile()
    return nc


def _trace(nc, tc, xt_d, xt8_d, wq8_d, hcat_d, wcat_d, bias_d, bv_d,
           vsel_d, cosq_d, sinq_d, cosk_d, sink_d, out_d):
    import contextlib
    ctx = contextlib.ExitStack()
    with ctx:
        consts = ctx.enter_context(tc.tile_pool(name="consts", bufs=1))
        acts = ctx.enter_context(tc.tile_pool(name="acts", bufs=1))
        qpool = ctx.enter_context(tc.tile_pool(name="qpool", bufs=4))
        wpool = ctx.enter_context(tc.tile_pool(name="wpool", bufs=2))
        swp = ctx.enter_context(tc.tile_pool(name="swp", bufs=4))
        q8p = ctx.enter_context(tc.tile_pool(name="q8p", bufs=2))
        sb512 = ctx.enter_context(tc.tile_pool(name="sb512", bufs=3))
        rcp_p = ctx.enter_context(tc.tile_pool(name="rcpp", bufs=1))
        f32p = ctx.enter_context(tc.tile_pool(name="f32p", bufs=2))
        psum = ctx.enter_context(tc.tile_pool(name="psum", bufs=4, space="PSUM"))
        pacc = ctx.enter_context(tc.tile_pool(name="pacc", bufs=2, space="PSUM"))

        def load_w(wname):
            wt = wpool.tile([P, TK, DIM], BF, tag="w", name=wname)
            nc.sync.dma_start(wt[:, :, :], wcat_d[_WIDX[wname], :, :, :])
            return wt

        # ---- DMAs in need order (first loads chunked per k-tile so the
        #      k-task matmuls can start as soon as slice 0 lands) ---------
        hcat_sb = consts.tile([P, TK, 2 * KV], BF, tag="hcat")
        nc.sync.dma_start(hcat_sb[:], hcat_d[:])
        bias_sb = consts.tile([P, 6, TK], F32, tag="bias")
        nc.sync.dma_start(bias_sb[:], bias_d[:])
        wkt = wpool.tile([P, TK, DIM], BF, tag="w", name="w_kt")
        for k in range(TK):
            nc.sync.dma_start(wkt[:, k, :], wcat_d[_WIDX["w_kt"], :, k, :])
        wq8a = q8p.tile([P, TK, DIM], F8D, tag="q8", name="wq8a")
        xt8_sb = acts.tile([P, BPC, TK, T], F8D, tag="xt8")
        for k in range(TK):
            nc.sync.dma_start(wq8a[:, k, :], wq8_d[0, :, k, :])
            nc.sync.dma_start(xt8_sb[:, 0, k, :], xt8_d[:, 0, k, :])
        cosq_sb = consts.tile([P, T], BF, tag="cosq")
        nc.sync.dma_start(cosq_sb[:], cosq_d[:])
        sinq_sb = consts.tile([P, T], BF, tag="sinq")
        nc.sync.dma_start(sinq_sb[:], sinq_d[:])
        wka = load_w("w_ka")
        nc.sync.dma_start(xt8_sb[:, 1], xt8_d[:, 1])
        wq8t = q8p.tile([P, TK, DIM], F8D, tag="q8", name="wq8t")
        nc.sync.dma_start(wq8t[:], wq8_d[1])
        cosk_sb = consts.tile([P, 2 * KV], BF, tag="cosk")
        nc.sync.dma_start(cosk_sb[:], cosk_d[:])
        sink_sb = consts.tile([P, 2 * KV], BF, tag="sink")
        nc.sync.dma_start(sink_sb[:], sink_d[:])
        bv_sb = consts.tile([P, DIM], BF, tag="bv")
        nc.sync.dma_start(bv_sb[:], bv_d[:])
        vsel = consts.tile([P, KV], BF, tag="vsel")
        nc.sync.dma_start(vsel[:], vsel_d[:])
        xt_sb = acts.tile([P, BPC, TK, T], BF, tag="xt")
        nc.sync.dma_start(xt_sb[:, 0], xt_d[:, 0])
        nc.sync.dma_start(xt_sb[:, 1], xt_d[:, 1])
        ones_mat = consts.tile([P, P], BF, tag="onesm")
        nc.vector.memset(ones_mat[:], 1.0)
        # scaled ones: softmax denominator picks up 1/OSC so the normalized
        # attention output comes out pre-scaled by OSC for the fp8 store
        sc_ones = consts.tile([P, P], BF, tag="sconem")
        nc.vector.memset(sc_ones[:], 1.0 / OSC)
        eps_sb = consts.tile([P, 1], F32, tag="eps")
        nc.vector.memset(eps_sb[:], LN_EPS * OSC * OSC)

        def bias_ap(bname, n):
            return bias_sb[:, _BIDX[bname], n:n + 1]

        def swap_halves(dst, sw, width):
            # rotate_half: swap the two 64-partition blocks via 2 DMAs,
            # issued from two idle queues so the issues overlap
            nc.gpsimd.dma_start(sw[0:64, 0:width], dst[64:128, 0:width])
            nc.sync.dma_start(sw[64:128, 0:width], dst[0:64, 0:width])

        def rope_q(dst):
            # dst: (128, T) bf16, in-place
            sw = swp.tile([P, T], BF, tag="sw")
            swap_halves(dst, sw[:, :], T)
            nc.vector.tensor_mul(sw[:], sw[:], sinq_sb[:])
            nc.vector.tensor_mul(dst, dst, cosq_sb[:])
            nc.vector.tensor_add(dst, dst, sw[:])

        DR = mybir.MatmulPerfMode.DoubleRow

        def q_mm(qt_t, w8, bname, b, n):
            # fp8 DoubleRow: contract 2 k-tiles per pass (K=256 virtual)
            for c in range(NCH):
                cs = slice(c * 512, (c + 1) * 512)
                ps = psum.tile([P, 512], F32, tag="ps")
                for kp in range(TK // 2):
                    nc.tensor.matmul(
                        ps[:], w8[:, 2 * kp:2 * kp + 2, n * P:(n + 1) * P],
                        xt8_sb[:, b, 2 * kp:2 * kp + 2, cs],
                        start=(kp == 0), stop=(kp == TK // 2 - 1),
                        perf_mode=DR)
                nc.scalar.activation(
                    qt_t[:, n, cs], ps[:], AF.Identity,
                    bias=bias_ap(bname, n), scale=QSCALE)
            rope_q(qt_t[:, n, :])

        # ================= k_task projection ===========================
        # krot columns: [0:64]=task b0, [64:128]=task b1, [128:130]=ad b0,
        # [130:132]=ad b1
        krot = acts.tile([P, TK, 2 * KV], BF, tag="krot")
        for n in range(TK):
            ps = psum.tile([P, 512], F32, tag="ps")
            for k in range(TK):
                nc.tensor.matmul(ps[:, 0:128], wkt[:, k, n * P:(n + 1) * P],
                                 hcat_sb[:, k, 0:128],
                                 start=(k == 0), stop=(k == TK - 1))
            nc.scalar.activation(krot[:, n, 0:128], ps[:, 0:128],
                                 AF.Identity, bias=bias_ap("b_kt", n), scale=1.0)

        # ================= q_adapter b0 ================================
        q_rot = {}  # (qi, b) -> (128, TK, T) bf16, qi: 0=adapter 1=task
        qa0 = qpool.tile([P, TK, T], BF, tag="qbuf", name="qa0")
        q_rot[(0, 0)] = qa0
        for n in range(TK):
            q_mm(qa0, wq8a, "b_qa", 0, n)

        # ================= k_adapter + k rope ==========================
        for n in range(TK):
            ps = psum.tile([P, 512], F32, tag="ps")
            for k in range(TK):
                nc.tensor.matmul(ps[:, 128:132], wka[:, k, n * P:(n + 1) * P],
                                 hcat_sb[:, k, 128:132],
                                 start=(k == 0), stop=(k == TK - 1))
            nc.scalar.activation(krot[:, n, 128:132], ps[:, 128:132],
                                 AF.Identity, bias=bias_ap("b_ka", n), scale=1.0)

        # ================= q_task b0 ===================================
        qt0 = qpool.tile([P, TK, T], BF, tag="qbuf", name="qt0")
        q_rot[(1, 0)] = qt0
        for n in range(TK):
            q_mm(qt0, wq8t, "b_qt", 0, n)

        # k rope (deferred: qt0's ropes gate T5, so they go first on DVE)
        for n in range(TK):
            sw = rcp_p.tile([P, 2 * KV], BF, tag="ksw")
            swap_halves(krot[:, n, :], sw[:, :], 2 * KV)
            nc.vector.tensor_mul(sw[:], sw[:], sink_sb[:])
            nc.vector.tensor_mul(krot[:, n, :], krot[:, n, :], cosk_sb[:])
            nc.vector.tensor_add(krot[:, n, :], krot[:, n, :], sw[:])

        wvt = load_w("w_vt")  # into slot freed by wkt
        wva = load_w("w_va")  # into slot freed by wka

        # ================= v projections (token-major) =================
        # vcomb rows: [0:64]=task tokens, [64:66]=adapter tokens; the bias
        # lands via a rank-2 matmul (vsel x bv) so the v pipeline only
        # depends on the PE + ScalarE (the qt0 rope tail drains under it)
        vcomb = acts.tile([P, BPC, DIM], BF, tag="vcomb")
        for b in range(BPC):
            for c in range(NCH):
                cs = slice(c * 512, (c + 1) * 512)
                ps = psum.tile([P, 512], F32, tag="ps")
                for k in range(TK):
                    nc.tensor.matmul(ps[0:64, :],
                                     hcat_sb[:, k, b * 64:(b + 1) * 64],
                                     wvt[:, k, cs],
                                     start=(k == 0), stop=(k == TK - 1))
                for k in range(TK):
                    nc.tensor.matmul(ps[64:66, :],
                                     hcat_sb[:, k, 128 + 2 * b:130 + 2 * b],
                                     wva[:, k, cs],
                                     start=(k == 0), stop=(k == TK - 1))
                nc.tensor.matmul(ps[0:KV, :], vsel[0:2, :], bv_sb[0:2, cs],
                                 start=False, stop=True, skip_group_check=True)
                nc.scalar.activation(vcomb[0:KV, b, cs], ps[0:KV, :],
                                     AF.Identity, scale=1.0)

        wo8 = q8p.tile([P, TK, DIM], F8D, tag="q8", name="wo8")
        nc.sync.dma_start(wo8[:], wq8_d[2])

        # ---- attention helpers ----------------------------------------
        attn = {}

        def attn_scores(b, h):
            out = []
            for c in range(NCH):
                cs = slice(c * 512, (c + 1) * 512)
                scps = psum.tile([P, 512], F32, tag="ps")
                nc.tensor.matmul(scps[0:64, :], krot[:, h, b * 64:(b + 1) * 64],
                                 q_rot[(1, b)][:, h, cs], start=True, stop=True)
                nc.tensor.matmul(scps[64:66, :],
                                 krot[:, h, 128 + 2 * b:130 + 2 * b],
                                 q_rot[(0, b)][:, h, cs], start=True, stop=True)
                ex = sb512.tile([P, 512], BF, tag="s", name="ex")
                nc.scalar.activation(ex[0:KV, :], scps[0:KV, :], AF.Exp)
                out.append(ex)
            return out

        def attn_finish(b, h, exs):
            at = attn[b]
            for c in range(NCH):
                cs = slice(c * 512, (c + 1) * 512)
                ex = exs[c]
                dnps = psum.tile([P, 512], F32, tag="ps")
                nc.tensor.matmul(dnps[:], sc_ones[0:KV, :],
                                 ex[0:KV, :], start=True, stop=True)
                ovps = psum.tile([P, 512], F32, tag="ps")
                nc.tensor.matmul(ovps[:], vcomb[0:KV, b, h * P:(h + 1) * P],
                                 ex[0:KV, :], start=True, stop=True)
                rcb = f32p.tile([P, 512], F32, tag="f32", name="rcb")
                nc.vector.reciprocal_approx_fast(rcb[:], dnps[:])
                nc.vector.tensor_mul(at[:, h, cs], ovps[:], rcb[:])

        # ================= q_adapter b1 ================================
        # (PE cover for the qt0 rope tail; its own ropes drain under T5)
        qa1 = qpool.tile([P, TK, T], BF, tag="qbuf", name="qa1")
        q_rot[(0, 1)] = qa1
        for n in range(TK):
            q_mm(qa1, wq8a, "b_qa", 1, n)

        # ============ T5: attention b0 (x) q_task b1 ===================
        attn[0] = acts.tile([P, TK, T], F8D, tag="attn0", name="attn0")
        qt1 = qpool.tile([P, TK, T], BF, tag="qbuf", name="qt1")
        q_rot[(1, 1)] = qt1
        prev = None
        for h in range(HEADS):
            exs = attn_scores(0, h)
            if prev is not None:
                attn_finish(0, h - 1, prev)
            prev = exs
            q_mm(qt1, wq8t, "b_qt", 1, h)
        attn_finish(0, HEADS - 1, prev)

        wffn = load_w("w_ffn")  # into slot freed by wva

        # ============ T6: o_proj b0 (x) attention b1 (lag-2) ===========
        def oproj_tile(b, n, y):
            # fp8 DoubleRow o_proj; psum = OSC*WS8*(attn@wo), so scale by
            # 1/WS8 and add OSC*(x + b_o) (b_o folded into x on the host)
            for c in range(NCH):
                cs = slice(c * 512, (c + 1) * 512)
                ps = psum.tile([P, 512], F32, tag="ps")
                for kp in range(TK // 2):
                    nc.tensor.matmul(
                        ps[:], wo8[:, 2 * kp:2 * kp + 2, n * P:(n + 1) * P],
                        attn[b][:, 2 * kp:2 * kp + 2, cs],
                        start=(kp == 0), stop=(kp == TK // 2 - 1),
                        perf_mode=DR)
                nc.vector.scalar_tensor_tensor(
                    y[:, n, cs], ps[:], 1.0 / WS8,
                    xt_sb[:, b, n, cs], OP.mult, OP.add)

        attn[1] = acts.tile([P, TK, T], F8D, tag="attn1", name="attn1")
        y0 = qpool.tile([P, TK, T], BF, tag="qbuf", name="y0")
        pend = {}
        for h in range(HEADS):
            oproj_tile(0, h, y0)
            if h >= 2:  # lag so the b1 q-rope tail drains under o_proj
                pend[h - 2] = attn_scores(1, h - 2)
            if h >= 3:
                attn_finish(1, h - 3, pend.pop(h - 3))
        for h in range(HEADS - 2, HEADS):
            pend[h] = attn_scores(1, h)
        for h in range(HEADS - 3, HEADS):
            attn_finish(1, h, pend.pop(h))

        # ---- layernorm (folded: yn = (y - mu) * rstd) -----------------
        def ln_stats_tile(sps, qps, y, n):
            for c in range(NCH):
                cs = slice(c * 512, (c + 1) * 512)
                ysq = sb512.tile([P, 512], BF, tag="s", name=f"ysq{n}{c}")
                nc.scalar.activation(ysq[:], y[:, n, cs], AF.Square)
                nc.tensor.matmul(sps[:, cs], ones_mat[:], y[:, n, cs],
                                 start=(n == 0), stop=(n == TK - 1),
                                 skip_group_check=True)
                nc.tensor.matmul(qps[:, cs], ones_mat[:], ysq[:],
                                 start=(n == 0), stop=(n == TK - 1),
                                 skip_group_check=True)

        def ln_apply(b, y, yn, sps, qps):
            # rstd_bc/mrs_bc: (128, T) bf16, broadcast over partitions
            rstd_bc = swp.tile([P, T], BF, tag="sw", name=f"rstd{b}")
            mrs_bc = swp.tile([P, T], BF, tag="sw", name=f"mrs{b}")
            for c in range(NCH):
                cs = slice(c * 512, (c + 1) * 512)
                mu = sb512.tile([P, 512], BF, tag="s", name=f"mu{b}{c}")
                nc.vector.tensor_scalar_mul(mu[:], sps[:, cs], 1.0 / DIM)
                m2 = sb512.tile([P, 512], BF, tag="s", name=f"m2{b}{c}")
                nc.vector.tensor_mul(m2[:], mu[:], mu[:])
                vq = f32p.tile([P, 512], F32, tag="f32")
                nc.vector.scalar_tensor_tensor(
                    vq[:], qps[:, cs], 1.0 / DIM, m2[:], OP.mult, OP.subtract)
                sdev = f32p.tile([P, 512], F32, tag="f32", name=f"sd{b}{c}")
                nc.scalar.activation(sdev[:], vq[:], AF.Sqrt, bias=eps_sb[:],
                                     scale=1.0)
                rstd_f = f32p.tile([P, 512], F32, tag="f32", name=f"rf{b}{c}")
                nc.vector.reciprocal_approx_fast(rstd_f[:], sdev[:])
                nc.vector.tensor_scalar_mul(rstd_bc[:, cs], rstd_f[:], 1.0)
                nc.vector.tensor_mul(mrs_bc[:, cs], mu[:], rstd_bc[:, cs])
            for n in range(TK):
                nc.vector.tensor_mul(yn[:, n, :], y[:, n, :], rstd_bc[:])
                nc.vector.tensor_tensor(yn[:, n, :], yn[:, n, :], mrs_bc[:],
                                        OP.subtract)

        def ffn_tile(b, n, yn):
            for c in range(NCH):
                cs = slice(c * 512, (c + 1) * 512)
                ps = psum.tile([P, 512], F32, tag="ps")
                for k in range(TK):
                    nc.tensor.matmul(ps[:], wffn[:, k, n * P:(n + 1) * P],
                                     yn[:, k, cs],
                                     start=(k == 0), stop=(k == TK - 1))
                ob = sb512.tile([P, 512], BF, tag="s", name=f"ob{b}{n}{c}")
                nc.scalar.activation(ob[:], ps[:], AF.Relu,
                                     bias=bias_ap("b_ffn", n), scale=1.0)
                nc.sync.dma_start(out_d[:, b, n, cs], ob[:])

        # ============ T7: LN b0 stats, o_proj b1, ln_apply(0) ==========
        y1 = qpool.tile([P, TK, T], BF, tag="qbuf", name="y1")
        sps0 = pacc.tile([P, T], F32, tag="acc", name="sps0")
        qps0 = pacc.tile([P, T], F32, tag="acc", name="qps0")
        for n in range(TK):
            ln_stats_tile(sps0, qps0, y0, n)
        for n in range(TK):
            oproj_tile(1, n, y1)
        yn0 = qpool.tile([P, TK, T], BF, tag="qbuf", name="yn0")
        ln_apply(0, y0, yn0, sps0, qps0)

        # ============ T8: LN b1 stats, ln_apply(1), ffn b0 =============
        sps1 = pacc.tile([P, T], F32, tag="acc", name="sps1")
        qps1 = pacc.tile([P, T], F32, tag="acc", name="qps1")
        for n in range(TK):
            ln_stats_tile(sps1, qps1, y1, n)
        yn1 = qpool.tile([P, TK, T], BF, tag="qbuf", name="yn1")
        ln_apply(1, y1, yn1, sps1, qps1)
        for n in range(TK):
            ffn_tile(0, n, yn0)

        # ============ T9: ffn b1 =======================================
        for n in range(TK):
            ffn_tile(1, n, yn1)


# =====================  host-side preparation  =========================

def _rope_tables(L):
    inv = 1.0 / (10000.0 ** (np.arange(0, HD, 2, dtype=np.float32) / HD))
    freqs = np.arange(L, dtype=np.float32)[:, None] * inv[None, :]
    emb = np.concatenate([freqs, freqs], axis=-1)  # (L, 128)
    return np.cos(emb), np.sin(emb)


def _perm_tables(L, scale):
    cos, sin = _rope_tables(L)  # (L, 128)
    cosP = (cos[:, _PERM_HEAD].T * scale).astype(np.float32)      # (128, L)
    sinN = (sin[:, _PERM_HEAD].T * _SIGN_HEAD[:, None] * scale).astype(np.float32)
    return cosP, sinN


def _w_sb(w, permute):
    # (1024 k, 1024 n) -> (128 p, 8 ko, 1024 n) bf16, optional column perm
    if permute:
        w = w[:, _PERM_FULL]
    return np.ascontiguousarray(
        w.reshape(TK, P, DIM).transpose(1, 0, 2)).astype(BF16)


def _b_slot(bvec, permute):
    if permute:
        bvec = bvec[_PERM_FULL]
    return bvec.reshape(TK, P).T  # (128, 8)


def kernel(**inputs):
    global _CACHED
    if _CACHED is None:
        _CACHED = _build_program()
    nc = _CACHED

    inp = {k: np.asarray(v) for k, v in inputs.items()}
    x = inp["x"].astype(np.float32)
    h_a = inp["h_a"].astype(np.float32)
    h_t = inp["h_t"].astype(np.float32)
    p_in = inp["p"].astype(np.float32)
    ratio = 1.0 / (1.0 + np.exp(-np.float32(inp["g"][0])))  # sigmoid

    # fold layernorm gamma/beta into the ffn weights
    w_ffn = inp["ln_g"].astype(np.float32)[:, None] * inp["w_ffn"].astype(np.float32)
    b_ffn = inp["b_ffn"].astype(np.float32) + (
        inp["ln_b"].astype(np.float32) @ inp["w_ffn"].astype(np.float32))

    # weights (shared across cores)
    wcat = np.stack([
        _w_sb(inp["w_qa"], True), _w_sb(inp["w_qt"], True),
        _w_sb(inp["w_ka"], True), _w_sb(inp["w_kt"], True),
        _w_sb(inp["w_va"], False), _w_sb(inp["w_vt"], False),
        _w_sb(inp["w_o"], False), _w_sb(w_ffn, False)])

    def _w8(w, permute):
        wp = w.astype(np.float32) * WS8
        if permute:
            wp = wp[:, _PERM_FULL]
        wp = np.clip(wp, -240.0, 240.0)
        return np.ascontiguousarray(
            wp.reshape(TK, P, DIM).transpose(1, 0, 2)).astype(FP8)

    wq8 = np.stack([_w8(inp["w_qa"], True), _w8(inp["w_qt"], True),
                    _w8(inp["w_o"], False)])
    bias_cat = np.stack([
        _b_slot(inp["b_qa"], True), _b_slot(inp["b_qt"], True),
        _b_slot(inp["b_ka"], True), _b_slot(inp["b_kt"], True),
        _b_slot(inp["b_o"], False) * 0.0, _b_slot(b_ffn, False)],
        axis=1).astype(np.float32)  # (128, 6slots, 8ko)
    bv_comb = np.zeros((P, DIM), np.float32)
    bv_comb[0, :] = inp["b_vt"]
    bv_comb[1, :] = inp["b_va"]
    bv_comb = bv_comb.astype(BF16)
    vsel = np.zeros((P, KV), np.float32)
    vsel[0, 0:KT] = 1.0
    vsel[1, KT:KV] = 1.0
    vsel = vsel.astype(BF16)

    cosq, sinq = _perm_tables(T, np.float32(1.0 / math.sqrt(HD)))
    coskt, sinkt = _perm_tables(KT, ratio)
    coska, sinka = _perm_tables(KA, np.float32(1.0))
    cosk = np.concatenate([coskt, coskt, coska, coska], axis=1)  # (128, 132)
    sink = np.concatenate([sinkt, sinkt, sinka, sinka], axis=1)

    shared = {
        "wcat": wcat, "wq8": wq8, "bias_cat": bias_cat, "bv_comb": bv_comb,
        "vsel": vsel,
        "cosq": cosq.astype(BF16), "sinq": sinq.astype(BF16),
        "cosk": cosk.astype(BF16), "sink": sink.astype(BF16),
    }

    in_maps = []
    for core in range(NCORES):
        b0 = core * BPC
        xc = x[b0:b0 + BPC]  # (2, 1024, 1024)
        xtf = np.ascontiguousarray(
            xc.reshape(BPC, T, TK, P).transpose(3, 0, 2, 1))  # (128,2,8,1024)
        xtr = xtf + inp["b_o"].astype(np.float32).reshape(TK, P).T[:, None, :, None]
        xt = (xtr * OSC).astype(BF16)  # o_proj residual: OSC*(x + b_o)
        xt8 = np.clip(xtf * XS8, -240.0, 240.0).astype(FP8)
        hcat = np.zeros((P, TK, 2 * KV), np.float32)
        for b in range(BPC):
            htT = h_t[b0 + b].T.reshape(TK, P, KT).transpose(1, 0, 2)
            hcat[:, :, b * KT:(b + 1) * KT] = htT
            had = np.stack([h_a[b0 + b, 0], p_in[b0 + b, 0]], axis=1)  # (1024,2)
            hcat[:, :, 2 * KT + b * KA:2 * KT + (b + 1) * KA] = (
                had.reshape(TK, P, KA).transpose(1, 0, 2))
        in_maps.append({"xt": xt, "xt8": xt8, "hcat": hcat.astype(BF16),
                        **shared})

    res = run_bass_kernel_spmd(nc, in_maps, core_ids=list(range(NCORES)))
    global LAST_RESULTS
    LAST_RESULTS = res

    out = np.empty((B, T, DIM), np.float32)
    for core in range(NCORES):
        ot = res.results[core]["outt"]  # (128, 2, 8, 1024) bf16
        out[core * BPC:(core + 1) * BPC] = (
            ot.astype(np.float32).transpose(1, 3, 2, 0).reshape(BPC, T, DIM))
    return out
